# revision 12
# baseline (speedup 1.0000x reference)
"""Trainium2 Bass kernel for nn_CombinedAMLModel (dense_mlp, 8 NeuronCores).

Sharding: tensor-parallel over the gene axis (20000 genes -> 2500 per core),
with the single collective being a ReduceScatter over the SAMPLES axis, so
the whole tail MLP runs locally per core (no AllReduce chain).

Per core:
  Phase A  - per-(tech,gene) 1->4->1 MLPs + per-gene tech combinor as 12
             relu-affine passes. Relu positive homogeneity folds |coe| into
             the ACT/DVE scale+bias on the host (c*relu(u) =
             sign(c)*relu(|c|u)), so PSUM accumulation uses host-precomputed
             fp16 sign-diagonal stationaries. Produces z[g_local, s]
             (126 x 1024 fp16, row 125 = ones for the layer-1 bias trick).
  Phase B  - out1 partial computed SAMPLES-ON-PARTITIONS: for each of 8
             sample tiles, psum[s,n] += z[g, s_tile].T @ CW0T[g, n] with the
             z slice stationary (weight-load amortized over 4 n-chunks).
             Row 125 of w0 carries Cb0/8 so the bias sums exactly in the RS.
  Phase C  - 4 chunked ReduceScatters over the samples axis (each 2 sample
             tiles), pipelined under phase B. Core c ends up owning samples
             q*256 + c*32 .. +32 for q in 0..3 (128 samples total).
  Phase D  - local tail 2000->200->20->1 on this core's 128 samples:
             PE transpose to put features on partitions, then 3 small
             matmul layers. No collective.

All matmuls run in fp16 (1 cycle/row on PE, ~7e-4 rel err end to end).
"""
import os
import sys

sys.path.insert(0, "/opt/trn_rl_repo")

import numpy as np
from contextlib import ExitStack

import concourse.bass as bass
import concourse.tile as tile
from concourse import bacc, mybir
from concourse.bass_utils import run_bass_kernel_spmd

T, S, G, H = 3, 1024, 20000, 4
NCORES = 8
GL = G // NCORES            # genes per core
PT = 125                    # gene-tile partition size
NGT = GL // PT              # gene tiles per core
NK = T * H                  # local relu-affine passes
KA = PT + 1                 # phase-B contraction rows (125 genes + ones row)
N1, N2, N3 = 2000, 200, 20
NST = 8                     # sample tiles (128 samples each)
NCH = 4                     # n-chunks per sample tile (500 each)
NW = N1 // NCH
SH = 512                    # PSUM-bank half of the sample axis for phase A
NQ = 4                      # ReduceScatter chunks (2 sample tiles each)
NKT = N1 // PT              # tail k-tiles (16)
ACT_KS = frozenset((0, 5, 10))  # passes on ScalarE; rest on VectorE

f32 = mybir.dt.float32
f16 = mybir.dt.float16

LAST_RUN = {}
_CACHE = {}


def _build_program():
    nc = bacc.Bacc("TRN2", target_bir_lowering=False, debug=False,
                   num_devices=NCORES)
    d = {}

    def inp(name, shape, dt=f32):
        d[name] = nc.dram_tensor(name, list(shape), dt, kind="ExternalInput").ap()

    inp("xT", (NGT, PT, T * S), f16)
    inp("scl", (PT, NGT * NK))
    inp("bia", (PT, NGT * NK))
    inp("cst", (PT, NGT))
    inp("sgncol", (PT, NGT * NK))
    inp("w0", (NGT, KA, N1), f16)
    inp("ident", (128, 64), f16)
    inp("identp", (PT, PT), f16)
    inp("onesr", (1, S), f16)
    inp("cw1t", (PT, NKT * N2), f16)
    inp("cb1", (100, 2))
    inp("cw2t", (100, 2 * N3), f16)
    inp("cb2", (N3, 1))
    inp("cwft", (N3, 1), f16)
    inp("cbf", (1, 1))
    out_d = nc.dram_tensor("out", [1, 128], f32, kind="ExternalOutput").ap()

    Relu = mybir.ActivationFunctionType.Relu
    Ident = mybir.ActivationFunctionType.Identity

    with tile.TileContext(nc) as tc, ExitStack() as ctx:
        const = ctx.enter_context(tc.tile_pool(name="const", bufs=1))
        xpool = ctx.enter_context(tc.tile_pool(name="x", bufs=6))
        dgpool = ctx.enter_context(tc.tile_pool(name="dg", bufs=6))
        apool = ctx.enter_context(tc.tile_pool(name="a", bufs=4))
        vpool = ctx.enter_context(tc.tile_pool(name="v", bufs=2))
        zpool = ctx.enter_context(tc.tile_pool(name="z", bufs=NGT))
        wpool = ctx.enter_context(tc.tile_pool(name="w0", bufs=NGT))
        opool = ctx.enter_context(tc.tile_pool(name="o1", bufs=2))
        tpool = ctx.enter_context(tc.tile_pool(name="tail", bufs=1))
        rpool = ctx.enter_context(tc.tile_pool(name="r1", bufs=NKT))
        zps = ctx.enter_context(tc.tile_pool(name="zps", bufs=4, space="PSUM"))
        bps = ctx.enter_context(tc.tile_pool(name="bps", bufs=4, space="PSUM"))
        dram = ctx.enter_context(tc.tile_pool(name="dram", bufs=1, space="DRAM"))

        # x preload for the first three gene tiles ahead of everything else.
        XQ = (nc.gpsimd, nc.sync, nc.scalar)
        x_pre = {}
        for gt in range(3):
            xt = xpool.tile([PT, T * S], f16, tag="x", name=f"x{gt}")
            XQ[gt % 3].dma_start(xt[:], d["xT"][gt])
            x_pre[gt] = xt

        identp = const.tile([PT, PT], f16)
        nc.scalar.dma_start(identp[:], d["identp"][:])
        sgnct = const.tile([PT, NGT * NK], f32)
        nc.scalar.dma_start(sgnct[:], d["sgncol"][:])
        sclt = const.tile([PT, NGT * NK], f32)
        nc.scalar.dma_start(sclt[:], d["scl"][:])
        identt = const.tile([128, 64], f16)
        nc.scalar.dma_start(identt[:], d["ident"][:])
        biat = const.tile([PT, NGT * NK], f32)
        nc.scalar.dma_start(biat[:], d["bia"][:])
        cstt = const.tile([PT, NGT], f32)
        nc.scalar.dma_start(cstt[:], d["cst"][:])
        cw1tt = const.tile([PT, NKT * N2], f16)
        nc.scalar.dma_start(cw1tt[:], d["cw1t"][:])
        cb1t = const.tile([100, 2], f32)
        nc.scalar.dma_start(cb1t[:], d["cb1"][:])
        cw2tt = const.tile([100, 2 * N3], f16)
        nc.scalar.dma_start(cw2tt[:], d["cw2t"][:])
        cb2t = const.tile([N3, 1], f32)
        nc.scalar.dma_start(cb2t[:], d["cb2"][:])
        cwftt = const.tile([N3, 1], f16)
        nc.scalar.dma_start(cwftt[:], d["cwft"][:])
        cbft = const.tile([1, 1], f32)
        nc.scalar.dma_start(cbft[:], d["cbf"][:])

        partial_c = [dram.tile([2, 128, N1], f16, tag=f"pc{q}",
                               name=f"partial{q}") for q in range(NQ)]
        rs_c = [dram.tile([32, N1], f16, tag=f"rs{q}",
                          name=f"rsout{q}") for q in range(NQ)]
        ccwarm_in = dram.tile([1, 128], f32, tag="ccwi")
        ccwarm_out = dram.tile([1, 128], f32, tag="ccwo")

        # tiny warm-up collective: absorbs the first-rendezvous / ncfw
        # cold-start cost during phase A instead of on the critical tail
        ccwarm_sb = const.tile([1, 128], f32)
        nc.vector.memset(ccwarm_sb[:], 0.0)
        nc.sync.dma_start(ccwarm_in[:], ccwarm_sb[:])
        nc.gpsimd.collective_compute(
            "AllReduce", mybir.AluOpType.add,
            replica_groups=[list(range(NCORES))],
            ins=[ccwarm_in.opt()], outs=[ccwarm_out.opt()],
        )

        # ---------------- Phase A: local gene MLPs + combinor ----------------
        z_tiles = []
        w_tiles = []
        for gt in range(NGT):
            if gt < 3:
                xt = x_pre[gt]
            else:
                xt = xpool.tile([PT, T * S], f16, tag="x", name=f"x{gt}")
                XQ[gt % 3].dma_start(xt[:], d["xT"][gt])
            xts = [xt[:, t * S:(t + 1) * S] for t in range(T)]
            # layer-1 weight block: resident for all of phase B
            w = wpool.tile([KA, N1], f16, tag="w0", name=f"w0_{gt}")
            (nc.sync if gt % 2 else nc.gpsimd).dma_start(w[:], d["w0"][gt])
            w_tiles.append(w)
            # z tile (row 125 = ones for the bias trick)
            z = zpool.tile([KA, S], f16, tag="z", name=f"z{gt}")
            nc.gpsimd.dma_start(z[PT:KA, :], d["onesr"][:])
            pss = (zps.tile([PT, SH], f32, tag="zps", name=f"zps{gt}_0"),
                   zps.tile([PT, SH], f32, tag="zps", name=f"zps{gt}_1"))
            for k in range(NK):
                t = k // H
                ci = gt * NK + k
                a = apool.tile([PT, S], f16, tag="a")
                if k in ACT_KS:
                    nc.scalar.activation(a[:], xts[t], Relu,
                                         bias=biat[:, ci:ci + 1],
                                         scale=sclt[:, ci:ci + 1])
                else:
                    # relu(s*x+b) = max(s*x, -b) + b; the +b is folded into
                    # cst on the host, so one DVE op per pass.
                    nc.vector.tensor_scalar(a[:], xts[t],
                                            sclt[:, ci:ci + 1],
                                            biat[:, ci:ci + 1],
                                            mybir.AluOpType.mult,
                                            mybir.AluOpType.max)
                dg = dgpool.tile([PT, PT], f16, tag="dg")
                nc.vector.tensor_scalar(dg[:], identp[:],
                                        sgnct[:, ci:ci + 1], None,
                                        mybir.AluOpType.mult)
                for sh in range(2):
                    nc.tensor.matmul(pss[sh][:], dg[:],
                                     a[:, sh * SH:(sh + 1) * SH],
                                     start=(k == 0), stop=(k == NK - 1))
            for sh in range(2):
                nc.scalar.activation(z[:PT, sh * SH:(sh + 1) * SH], pss[sh][:],
                                     Ident, bias=cstt[:, gt:gt + 1], scale=1.0)
            z_tiles.append(z)

        # ------- Phase B: out1[s, n] partials, samples on partitions --------
        # z slice stationary (one weight load per gene tile, 4 moving chunks);
        # each 2-sample-tile chunk ReduceScatters as soon as it is stored.
        for st in range(NST):
            o = opool.tile([128, N1], f16, tag="o1")
            pps = [bps.tile([128, NW], f32, tag="bps", name=f"bp{st}_{j}")
                   for j in range(NCH)]
            for gt in range(NGT):
                zsl = z_tiles[gt][:, st * 128:(st + 1) * 128]
                for j in range(NCH):
                    nc.tensor.matmul(pps[j][:], zsl,
                                     w_tiles[gt][:, j * NW:(j + 1) * NW],
                                     start=(gt == 0), stop=(gt == NGT - 1))
            q, half = divmod(st, 2)
            for j in range(NCH):
                nc.scalar.copy(o[:, j * NW:(j + 1) * NW], pps[j][:])
                nc.sync.dma_start(
                    partial_c[q][half, :, j * NW:(j + 1) * NW],
                    o[:, j * NW:(j + 1) * NW])
            if half == 1:
                nc.gpsimd.collective_compute(
                    "ReduceScatter", mybir.AluOpType.add,
                    replica_groups=[list(range(NCORES))],
                    ins=[partial_c[q].opt()], outs=[rs_c[q].opt()],
                )

        # ------- Phase D: local tail 2000 -> 200 -> 20 -> 1 ------------------
        # pipelined per 32-sample RS chunk: relu+transpose+layer2 columns for
        # chunk q run as soon as rs_c[q] lands, hiding q<3 under later RSes.
        y1 = tpool.tile([128, N1], f16, tag="y1")
        yb = tpool.tile([128, N1], f16, tag="yb")
        r1_tiles = [rpool.tile([PT, 128], f16, tag="r1", name=f"r1_{kt}")
                    for kt in range(NKT)]
        ps2s = [bps.tile([100, 128], f32, tag="bps", name=f"ps2_{mc}")
                for mc in range(2)]
        for q in range(NQ):
            rows = slice(q * 32, (q + 1) * 32)
            (nc.scalar if q % 2 else nc.sync).dma_start(y1[rows, :], rs_c[q][:])
            nc.scalar.activation(yb[rows, :], y1[rows, :], Relu)
        for h in range(2):
            rows = slice(h * 64, (h + 1) * 64)
            for kt in range(NKT):
                tp = zps.tile([PT, 64], f16, tag="zps", name=f"tp{h}_{kt}")
                nc.tensor.transpose(tp[:], yb[rows, kt * PT:(kt + 1) * PT],
                                    identt[rows, :])
                if kt % 2:
                    nc.scalar.copy(r1_tiles[kt][:, rows], tp[:])
                else:
                    nc.vector.tensor_copy(r1_tiles[kt][:, rows], tp[:])
            for mc in range(2):
                for kt in range(NKT):
                    nc.tensor.matmul(
                        ps2s[mc][:, rows],
                        cw1tt[:, kt * N2 + mc * 100:kt * N2 + (mc + 1) * 100],
                        r1_tiles[kt][:, rows],
                        start=(kt == 0), stop=(kt == NKT - 1))
        z2 = tpool.tile([100, 2 * 128], f16, tag="z2")
        for mc in range(2):
            nc.scalar.activation(z2[:, mc * 128:(mc + 1) * 128], ps2s[mc][:],
                                 Relu, bias=cb1t[:, mc:mc + 1], scale=1.0)
        ps3 = bps.tile([N3, 128], f32, tag="bps", name="ps3")
        for mc in range(2):
            nc.tensor.matmul(ps3[:], cw2tt[:, mc * N3:(mc + 1) * N3],
                             z2[:, mc * 128:(mc + 1) * 128],
                             start=(mc == 0), stop=(mc == 1))
        z3 = tpool.tile([N3, 128], f16, tag="z3")
        nc.scalar.activation(z3[:], ps3[:], Relu, bias=cb2t[:], scale=1.0)
        psf = bps.tile([1, 128], f32, tag="bps", name="psf")
        nc.tensor.matmul(psf[:], cwftt[:], z3[:], start=True, stop=True)
        outt = tpool.tile([1, 128], f32, tag="outt")
        nc.scalar.activation(outt[:], psf[:], Ident, bias=cbft[:], scale=1.0)
        nc.sync.dma_start(out_d[:], outt[:])

    nc.compile()
    return nc


def _shard_inputs(x, W1, b1, W2, b2, Wc, bc,
                  CW0, Cb0, CW1, Cb1, CW2, Cb2, CWf, Cbf):
    f = lambda a: np.ascontiguousarray(a, dtype=np.float32)
    h16 = lambda a: np.ascontiguousarray(a).astype(np.float16)
    # tail weights: CW1T packed (125, 16*200); kt-major columns
    cw1t = h16(CW1.T.reshape(NKT, PT, N2).transpose(1, 0, 2).reshape(PT, NKT * N2))
    shared = {
        "ident": np.tile(np.eye(64, dtype=np.float16), (2, 1)),
        "identp": np.eye(PT, dtype=np.float16),
        "onesr": np.ones((1, S), dtype=np.float16),
        "cw1t": cw1t,
        "cb1": f(Cb1.reshape(2, 100).T),
        "cw2t": h16(CW2.T.reshape(2, 100, N3).transpose(1, 0, 2).reshape(100, 2 * N3)),
        "cb2": f(Cb2.reshape(N3, 1)),
        "cwft": h16(CWf.T),
        "cbf": f(Cbf.reshape(1, 1)),
    }
    in_maps = []
    for c in range(NCORES):
        gs = slice(c * GL, (c + 1) * GL)
        coe = (W2[:, gs, :] * Wc[gs, :].T[:, :, None]) \
            .transpose(1, 0, 2).reshape(GL, NK)
        aco = np.abs(coe)
        scl = aco * W1[:, gs, :].transpose(1, 0, 2).reshape(GL, NK)
        bia = aco * b1[:, gs, :].transpose(1, 0, 2).reshape(GL, NK)
        sgv = np.sign(coe)
        cst = (b2[:, gs] * Wc[gs, :].T).sum(0) + bc[gs]
        # DVE passes emit max(s*x,-b); fold their sgn*b into the constant,
        # and send -b as the bias for those columns.
        dve_ks = np.array([k not in ACT_KS for k in range(NK)])
        cst = cst + (sgv * bia)[:, dve_ks].sum(1)
        bia = np.where(dve_ks[None, :], -bia, bia)
        # layer-1 weights: (NGT, 126, N1), row 125 = Cb0/8 on gt 0
        w0 = np.zeros((NGT, KA, N1), dtype=np.float16)
        w0[:, :PT, :] = CW0[:, gs].T.reshape(NGT, PT, N1)
        w0[0, PT, :] = (Cb0 / NCORES).astype(np.float16)
        in_maps.append({
            "xT": h16(x[:, :, gs].transpose(2, 0, 1).reshape(NGT, PT, T * S)),
            "scl": f(scl.reshape(NGT, PT, NK).transpose(1, 0, 2)
                     .reshape(PT, NGT * NK)),
            "bia": f(bia.reshape(NGT, PT, NK).transpose(1, 0, 2)
                     .reshape(PT, NGT * NK)),
            "cst": f(cst.reshape(NGT, PT).T),
            "sgncol": f(sgv.reshape(NGT, PT, NK).transpose(1, 0, 2)
                        .reshape(PT, NGT * NK)),
            "w0": w0,
            **shared,
        })
    return in_maps


def _install_profile_shim():
    """Register the NTFF profiling hook that this container's antenv lacks.

    bass_utils' trace path imports antenv.axon_hooks; the boot helper that
    can construct the actual hook exists, so wire it up dynamically.
    """
    import types
    try:
        import antenv.axon_hooks  # noqa: F401
        return True
    except ImportError:
        pass
    try:
        import antenv
        from trn_agent_boot.trn_boot import _ntff_profile_via_ctypes
        hook = _ntff_profile_via_ctypes("/opt/axon/libaxon_pjrt.so")
        mod = types.ModuleType("antenv.axon_hooks")
        mod.get_axon_ntff_profile_hook = lambda: hook
        mod.set_axon_ntff_profile_hook = lambda h: None
        sys.modules["antenv.axon_hooks"] = mod
        antenv.axon_hooks = mod
        return hook is not None
    except Exception:
        return False


def kernel(**inputs):
    inputs = {k: np.asarray(v) for k, v in inputs.items()}
    in_maps = _shard_inputs(**inputs)
    if "nc" not in _CACHE:
        _CACHE["nc"] = _build_program()
    nc = _CACHE["nc"]
    trace = bool(os.environ.get("KERNEL_PROFILE")) and _install_profile_shim()
    res = run_bass_kernel_spmd(nc, in_maps, core_ids=list(range(NCORES)),
                               trace=trace)
    LAST_RUN["exec_time_ns"] = res.exec_time_ns
    LAST_RUN["mean_exec_time_ns"] = res.mean_exec_time_ns
    if res.instructions_and_trace is not None:
        LAST_RUN["trace_path"] = res.instructions_and_trace[1]
    # core c owns samples q*256 + c*32 .. +32 for q in 0..3
    out = np.empty((1, S, 1), dtype=np.float32)
    for c in range(NCORES):
        oc = np.asarray(res.results[c]["out"]).reshape(128)
        for q in range(NQ):
            out[0, q * 256 + c * 32:q * 256 + (c + 1) * 32, 0] = \
                oc[q * 32:(q + 1) * 32]
    return out


if __name__ == "__main__":
    rng = np.random.default_rng(0)
    ins = {
        "x": rng.standard_normal((T, S, G), dtype=np.float32),
        "W1": rng.standard_normal((T, G, H), dtype=np.float32) * 0.5,
        "b1": rng.standard_normal((T, G, H), dtype=np.float32) * 0.1,
        "W2": rng.standard_normal((T, G, H), dtype=np.float32) * 0.5,
        "b2": rng.standard_normal((T, G), dtype=np.float32) * 0.1,
        "Wc": rng.standard_normal((G, T), dtype=np.float32) * 0.5,
        "bc": rng.standard_normal((G,), dtype=np.float32) * 0.1,
        "CW0": rng.standard_normal((N1, G), dtype=np.float32) * 0.007,
        "Cb0": rng.standard_normal((N1,), dtype=np.float32) * 0.007,
        "CW1": rng.standard_normal((N2, N1), dtype=np.float32) * 0.02,
        "Cb1": rng.standard_normal((N2,), dtype=np.float32) * 0.02,
        "CW2": rng.standard_normal((N3, N2), dtype=np.float32) * 0.07,
        "Cb2": rng.standard_normal((N3,), dtype=np.float32) * 0.07,
        "CWf": rng.standard_normal((1, N3), dtype=np.float32) * 0.2,
        "Cbf": rng.standard_normal((1,), dtype=np.float32) * 0.2,
    }
    out = kernel(**ins)
    # numpy reference
    xx = ins["x"]
    h = np.maximum(xx[..., None] * ins["W1"][:, None] + ins["b1"][:, None], 0.0)
    y = np.einsum("tsgh,tgh->tsg", h, ins["W2"]) + ins["b2"][:, None, :]
    zz = np.einsum("tsg,gt->sg", y, ins["Wc"]) + ins["bc"]
    for Wl, bl in ((ins["CW0"], ins["Cb0"]), (ins["CW1"], ins["Cb1"]),
                   (ins["CW2"], ins["Cb2"])):
        zz = np.maximum(zz @ Wl.T + bl, 0.0)
    ref = (zz @ ins["CWf"].T + ins["Cbf"])[None]
    err = np.abs(out - ref).max() / (np.abs(ref).max() + 1e-12)
    print("self-test rel err:", err)
    print("exec_time_ns:", LAST_RUN.get("exec_time_ns"))


# revision 15
# speedup vs baseline: 1.0162x; 1.0162x over previous
"""Trainium2 Bass kernel for nn_CombinedAMLModel (dense_mlp, 8 NeuronCores).

Sharding: tensor-parallel over the gene axis (20000 genes -> 2500 per core),
with the single collective being a ReduceScatter over the SAMPLES axis, so
the whole tail MLP runs locally per core (no AllReduce chain).

Per core:
  Phase A  - per-(tech,gene) 1->4->1 MLPs + per-gene tech combinor as 12
             relu-affine passes. Relu positive homogeneity folds |coe| into
             the ACT/DVE scale+bias on the host (c*relu(u) =
             sign(c)*relu(|c|u)), so PSUM accumulation uses host-precomputed
             fp16 sign-diagonal stationaries. Produces z[g_local, s]
             (126 x 1024 fp16, row 125 = ones for the layer-1 bias trick).
  Phase B  - out1 partial computed SAMPLES-ON-PARTITIONS: for each of 8
             sample tiles, psum[s,n] += z[g, s_tile].T @ CW0T[g, n] with the
             z slice stationary (weight-load amortized over 4 n-chunks).
             Row 125 of w0 carries Cb0/8 so the bias sums exactly in the RS.
  Phase C  - 4 chunked ReduceScatters over the samples axis (each 2 sample
             tiles), pipelined under phase B. Core c ends up owning samples
             q*256 + c*32 .. +32 for q in 0..3 (128 samples total).
  Phase D  - local tail 2000->200->20->1 on this core's 128 samples:
             PE transpose to put features on partitions, then 3 small
             matmul layers. No collective.

All matmuls run in fp16 (1 cycle/row on PE, ~7e-4 rel err end to end).
"""
import os
import sys

sys.path.insert(0, "/opt/trn_rl_repo")

import numpy as np
from contextlib import ExitStack

import concourse.bass as bass
import concourse.tile as tile
from concourse import bacc, mybir
from concourse.bass_utils import run_bass_kernel_spmd

T, S, G, H = 3, 1024, 20000, 4
NCORES = 8
GL = G // NCORES            # genes per core
PT = 125                    # gene-tile partition size
NGT = GL // PT              # gene tiles per core
NK = T * H                  # local relu-affine passes
KA = PT + 1                 # phase-B contraction rows (125 genes + ones row)
N1, N2, N3 = 2000, 200, 20
NST = 8                     # sample tiles (128 samples each)
NCH = 4                     # n-chunks per sample tile (500 each)
NW = N1 // NCH
SH = 512                    # PSUM-bank half of the sample axis for phase A
NQ = 4                      # ReduceScatter chunks (2 sample tiles each)
NKT = N1 // PT              # tail k-tiles (16)
ACT_KS = frozenset((0, 5, 10))  # passes on ScalarE; rest on VectorE

f32 = mybir.dt.float32
f16 = mybir.dt.float16

LAST_RUN = {}
_CACHE = {}


def _build_program():
    nc = bacc.Bacc("TRN2", target_bir_lowering=False, debug=False,
                   num_devices=NCORES)
    d = {}

    def inp(name, shape, dt=f32):
        d[name] = nc.dram_tensor(name, list(shape), dt, kind="ExternalInput").ap()

    inp("xT", (NGT, PT, T * S), f16)
    # cpack: [sgncol | scl | bia | cst] along the free axis, fp32
    inp("cpack", (PT, 3 * NGT * NK + NGT))
    # fpack: [identp | cw1t] fp16
    inp("fpack", (PT, PT + NKT * N2), f16)
    inp("w0", (KA, NGT * N1), f16)
    inp("ident", (128, 64), f16)
    inp("onesr", (1, NGT * S), f16)
    inp("cb1", (100, 2))
    inp("cw2t", (100, 2 * N3), f16)
    inp("cb2", (N3, 1))
    inp("cwft", (N3, 1), f16)
    inp("cbf", (1, 1))
    out_d = nc.dram_tensor("out", [1, 128], f32, kind="ExternalOutput").ap()

    Relu = mybir.ActivationFunctionType.Relu
    Ident = mybir.ActivationFunctionType.Identity

    with tile.TileContext(nc) as tc, ExitStack() as ctx:
        const = ctx.enter_context(tc.tile_pool(name="const", bufs=1))
        xpool = ctx.enter_context(tc.tile_pool(name="x", bufs=6))
        dgpool = ctx.enter_context(tc.tile_pool(name="dg", bufs=6))
        apool = ctx.enter_context(tc.tile_pool(name="a", bufs=4))
        vpool = ctx.enter_context(tc.tile_pool(name="v", bufs=2))
        zpool = ctx.enter_context(tc.tile_pool(name="z", bufs=1))
        wpool = ctx.enter_context(tc.tile_pool(name="w0", bufs=1))
        opool = ctx.enter_context(tc.tile_pool(name="o1", bufs=2))
        tpool = ctx.enter_context(tc.tile_pool(name="tail", bufs=1))
        rpool = ctx.enter_context(tc.tile_pool(name="r1", bufs=NKT))
        zps = ctx.enter_context(tc.tile_pool(name="zps", bufs=4, space="PSUM"))
        bps = ctx.enter_context(tc.tile_pool(name="bps", bufs=4, space="PSUM"))
        dram = ctx.enter_context(tc.tile_pool(name="dram", bufs=1, space="DRAM"))

        # x preload for the first three gene tiles ahead of everything else.
        XQ = (nc.gpsimd, nc.sync)
        x_pre = {}
        for gt in range(3):
            xt = xpool.tile([PT, T * S], f16, tag="x", name=f"x{gt}")
            XQ[gt % 2].dma_start(xt[:], d["xT"][gt])
            x_pre[gt] = xt

        # z and w0 as single big tiles: one ones-row DMA, four w0 DMAs
        zbig = zpool.tile([KA, NGT * S], f16, tag="z", name="zbig")
        nc.gpsimd.dma_start(zbig[PT:KA, :], d["onesr"][:])
        wbig = wpool.tile([KA, NGT * N1], f16, tag="w0", name="wbig")

        NKK = NGT * NK
        cpackt = const.tile([PT, 3 * NKK + NGT], f32)
        nc.scalar.dma_start(cpackt[:], d["cpack"][:])
        sgnct = cpackt[:, 0:NKK]
        sclt = cpackt[:, NKK:2 * NKK]
        biat = cpackt[:, 2 * NKK:3 * NKK]
        cstt = cpackt[:, 3 * NKK:3 * NKK + NGT]
        fpackt = const.tile([PT, PT + NKT * N2], f16)
        nc.scalar.dma_start(fpackt[:], d["fpack"][:])
        identp = fpackt[:, 0:PT]
        cw1tt = fpackt[:, PT:]
        identt = const.tile([128, 64], f16)
        nc.scalar.dma_start(identt[:], d["ident"][:])
        cb1t = const.tile([100, 2], f32)
        nc.scalar.dma_start(cb1t[:], d["cb1"][:])
        cw2tt = const.tile([100, 2 * N3], f16)
        nc.scalar.dma_start(cw2tt[:], d["cw2t"][:])
        cb2t = const.tile([N3, 1], f32)
        nc.scalar.dma_start(cb2t[:], d["cb2"][:])
        cwftt = const.tile([N3, 1], f16)
        nc.scalar.dma_start(cwftt[:], d["cwft"][:])
        cbft = const.tile([1, 1], f32)
        nc.scalar.dma_start(cbft[:], d["cbf"][:])

        partial_c = [dram.tile([2, 128, N1], f16, tag=f"pc{q}",
                               name=f"partial{q}") for q in range(NQ)]
        rs_c = [dram.tile([32, N1], f16, tag=f"rs{q}",
                          name=f"rsout{q}") for q in range(NQ)]
        ccwarm_in = dram.tile([1, 128], f32, tag="ccwi")
        ccwarm_out = dram.tile([1, 128], f32, tag="ccwo")

        # tiny warm-up collective: absorbs the first-rendezvous / ncfw
        # cold-start cost during phase A instead of on the critical tail
        ccwarm_sb = const.tile([1, 128], f32)
        nc.vector.memset(ccwarm_sb[:], 0.0)
        nc.sync.dma_start(ccwarm_in[:], ccwarm_sb[:])
        nc.gpsimd.collective_compute(
            "AllReduce", mybir.AluOpType.add,
            replica_groups=[list(range(NCORES))],
            ins=[ccwarm_in.opt()], outs=[ccwarm_out.opt()],
        )

        # ---------------- Phase A: local gene MLPs + combinor ----------------
        z_tiles = []
        w_tiles = []
        for gt in range(NGT):
            if gt < 3:
                xt = x_pre[gt]
            else:
                xt = xpool.tile([PT, T * S], f16, tag="x", name=f"x{gt}")
                XQ[gt % 2].dma_start(xt[:], d["xT"][gt])
            if gt % 5 == 3:          # w0 quarters on the idle scalar queue
                wc = gt // 5
                cols = slice(wc * 5 * N1, (wc + 1) * 5 * N1)
                nc.scalar.dma_start(wbig[:, cols], d["w0"][:, cols])
            xts = [xt[:, t * S:(t + 1) * S] for t in range(T)]
            z = zbig[:, gt * S:(gt + 1) * S]
            w_tiles.append(wbig[:, gt * N1:(gt + 1) * N1])
            pss = (zps.tile([PT, SH], f32, tag="zps", name=f"zps{gt}_0"),
                   zps.tile([PT, SH], f32, tag="zps", name=f"zps{gt}_1"))
            for k in range(NK):
                t = k // H
                ci = gt * NK + k
                a = apool.tile([PT, S], f16, tag="a")
                if k in ACT_KS:
                    nc.scalar.activation(a[:], xts[t], Relu,
                                         bias=biat[:, ci:ci + 1],
                                         scale=sclt[:, ci:ci + 1])
                else:
                    # relu(s*x+b) = max(s*x, -b) + b; the +b is folded into
                    # cst on the host, so one DVE op per pass.
                    nc.vector.tensor_scalar(a[:], xts[t],
                                            sclt[:, ci:ci + 1],
                                            biat[:, ci:ci + 1],
                                            mybir.AluOpType.mult,
                                            mybir.AluOpType.max)
                dg = dgpool.tile([PT, PT], f16, tag="dg")
                nc.vector.tensor_scalar(dg[:], identp[:],
                                        sgnct[:, ci:ci + 1], None,
                                        mybir.AluOpType.mult)
                for sh in range(2):
                    nc.tensor.matmul(pss[sh][:], dg[:],
                                     a[:, sh * SH:(sh + 1) * SH],
                                     start=(k == 0), stop=(k == NK - 1))
            for sh in range(2):
                nc.scalar.activation(z[:PT, sh * SH:(sh + 1) * SH], pss[sh][:],
                                     Ident, bias=cstt[:, gt:gt + 1], scale=1.0)
            z_tiles.append(z)

        # ------- Phase B: out1[s, n] partials, samples on partitions --------
        # z slice stationary (one weight load per gene tile, 4 moving chunks);
        # each 2-sample-tile chunk ReduceScatters as soon as it is stored.
        for st in range(NST):
            o = opool.tile([128, N1], f16, tag="o1")
            pps = [bps.tile([128, NW], f32, tag="bps", name=f"bp{st}_{j}")
                   for j in range(NCH)]
            for gt in range(NGT):
                zsl = z_tiles[gt][:, st * 128:(st + 1) * 128]
                for j in range(NCH):
                    nc.tensor.matmul(pps[j][:], zsl,
                                     w_tiles[gt][:, j * NW:(j + 1) * NW],
                                     start=(gt == 0), stop=(gt == NGT - 1))
            q, half = divmod(st, 2)
            for j in range(NCH):
                nc.scalar.copy(o[:, j * NW:(j + 1) * NW], pps[j][:])
                nc.sync.dma_start(
                    partial_c[q][half, :, j * NW:(j + 1) * NW],
                    o[:, j * NW:(j + 1) * NW])
            if half == 1:
                nc.gpsimd.collective_compute(
                    "ReduceScatter", mybir.AluOpType.add,
                    replica_groups=[list(range(NCORES))],
                    ins=[partial_c[q].opt()], outs=[rs_c[q].opt()],
                )

        # ------- Phase D: local tail 2000 -> 200 -> 20 -> 1 ------------------
        # pipelined per 32-sample RS chunk: relu+transpose+layer2 columns for
        # chunk q run as soon as rs_c[q] lands, hiding q<3 under later RSes.
        y1 = tpool.tile([128, N1], f16, tag="y1")
        yb = tpool.tile([128, N1], f16, tag="yb")
        r1_tiles = [rpool.tile([PT, 128], f16, tag="r1", name=f"r1_{kt}")
                    for kt in range(NKT)]
        ps2s = [bps.tile([100, 128], f32, tag="bps", name=f"ps2_{mc}")
                for mc in range(2)]
        for q in range(NQ):
            rows = slice(q * 32, (q + 1) * 32)
            (nc.scalar if q % 2 else nc.sync).dma_start(y1[rows, :], rs_c[q][:])
            nc.scalar.activation(yb[rows, :], y1[rows, :], Relu)
        for h in range(2):
            rows = slice(h * 64, (h + 1) * 64)
            for kt in range(NKT):
                tp = zps.tile([PT, 64], f16, tag="zps", name=f"tp{h}_{kt}")
                nc.tensor.transpose(tp[:], yb[rows, kt * PT:(kt + 1) * PT],
                                    identt[rows, :])
                if kt % 2:
                    nc.scalar.copy(r1_tiles[kt][:, rows], tp[:])
                else:
                    nc.vector.tensor_copy(r1_tiles[kt][:, rows], tp[:])
            for mc in range(2):
                for kt in range(NKT):
                    nc.tensor.matmul(
                        ps2s[mc][:, rows],
                        cw1tt[:, kt * N2 + mc * 100:kt * N2 + (mc + 1) * 100],
                        r1_tiles[kt][:, rows],
                        start=(kt == 0), stop=(kt == NKT - 1))
        z2 = tpool.tile([100, 2 * 128], f16, tag="z2")
        for mc in range(2):
            nc.scalar.activation(z2[:, mc * 128:(mc + 1) * 128], ps2s[mc][:],
                                 Relu, bias=cb1t[:, mc:mc + 1], scale=1.0)
        ps3 = bps.tile([N3, 128], f32, tag="bps", name="ps3")
        for mc in range(2):
            nc.tensor.matmul(ps3[:], cw2tt[:, mc * N3:(mc + 1) * N3],
                             z2[:, mc * 128:(mc + 1) * 128],
                             start=(mc == 0), stop=(mc == 1))
        z3 = tpool.tile([N3, 128], f16, tag="z3")
        nc.scalar.activation(z3[:], ps3[:], Relu, bias=cb2t[:], scale=1.0)
        psf = bps.tile([1, 128], f32, tag="bps", name="psf")
        nc.tensor.matmul(psf[:], cwftt[:], z3[:], start=True, stop=True)
        outt = tpool.tile([1, 128], f32, tag="outt")
        nc.scalar.activation(outt[:], psf[:], Ident, bias=cbft[:], scale=1.0)
        nc.sync.dma_start(out_d[:], outt[:])

    nc.compile()
    return nc


def _shard_inputs(x, W1, b1, W2, b2, Wc, bc,
                  CW0, Cb0, CW1, Cb1, CW2, Cb2, CWf, Cbf):
    f = lambda a: np.ascontiguousarray(a, dtype=np.float32)
    h16 = lambda a: np.ascontiguousarray(a).astype(np.float16)
    # tail weights: CW1T packed (125, 16*200); kt-major columns
    cw1t = h16(CW1.T.reshape(NKT, PT, N2).transpose(1, 0, 2).reshape(PT, NKT * N2))
    shared = {
        "ident": np.tile(np.eye(64, dtype=np.float16), (2, 1)),
        "fpack": np.concatenate([np.eye(PT, dtype=np.float16), cw1t], axis=1),
        "onesr": np.ones((1, NGT * S), dtype=np.float16),
        "cb1": f(Cb1.reshape(2, 100).T),
        "cw2t": h16(CW2.T.reshape(2, 100, N3).transpose(1, 0, 2).reshape(100, 2 * N3)),
        "cb2": f(Cb2.reshape(N3, 1)),
        "cwft": h16(CWf.T),
        "cbf": f(Cbf.reshape(1, 1)),
    }
    in_maps = []
    for c in range(NCORES):
        gs = slice(c * GL, (c + 1) * GL)
        coe = (W2[:, gs, :] * Wc[gs, :].T[:, :, None]) \
            .transpose(1, 0, 2).reshape(GL, NK)
        aco = np.abs(coe)
        scl = aco * W1[:, gs, :].transpose(1, 0, 2).reshape(GL, NK)
        bia = aco * b1[:, gs, :].transpose(1, 0, 2).reshape(GL, NK)
        sgv = np.sign(coe)
        cst = (b2[:, gs] * Wc[gs, :].T).sum(0) + bc[gs]
        # DVE passes emit max(s*x,-b); fold their sgn*b into the constant,
        # and send -b as the bias for those columns.
        dve_ks = np.array([k not in ACT_KS for k in range(NK)])
        cst = cst + (sgv * bia)[:, dve_ks].sum(1)
        bia = np.where(dve_ks[None, :], -bia, bia)
        # layer-1 weights: (126, NGT*N1), row 125 = Cb0/8 on gt 0
        w0 = np.zeros((KA, NGT * N1), dtype=np.float16)
        w0[:PT, :] = CW0[:, gs].T.reshape(NGT, PT, N1) \
            .transpose(1, 0, 2).reshape(PT, NGT * N1)
        w0[PT, :N1] = (Cb0 / NCORES).astype(np.float16)
        pk = lambda a: a.reshape(NGT, PT, NK).transpose(1, 0, 2).reshape(PT, NGT * NK)
        cpack = np.concatenate(
            [pk(sgv), pk(scl), pk(bia), cst.reshape(NGT, PT).T], axis=1)
        in_maps.append({
            "xT": h16(x[:, :, gs].transpose(2, 0, 1).reshape(NGT, PT, T * S)),
            "cpack": f(cpack),
            "w0": w0,
            **shared,
        })
    return in_maps


def _install_profile_shim():
    """Register the NTFF profiling hook that this container's antenv lacks.

    bass_utils' trace path imports antenv.axon_hooks; the boot helper that
    can construct the actual hook exists, so wire it up dynamically.
    """
    import types
    try:
        import antenv.axon_hooks  # noqa: F401
        return True
    except ImportError:
        pass
    try:
        import antenv
        from trn_agent_boot.trn_boot import _ntff_profile_via_ctypes
        hook = _ntff_profile_via_ctypes("/opt/axon/libaxon_pjrt.so")
        mod = types.ModuleType("antenv.axon_hooks")
        mod.get_axon_ntff_profile_hook = lambda: hook
        mod.set_axon_ntff_profile_hook = lambda h: None
        sys.modules["antenv.axon_hooks"] = mod
        antenv.axon_hooks = mod
        return hook is not None
    except Exception:
        return False


def kernel(**inputs):
    inputs = {k: np.asarray(v) for k, v in inputs.items()}
    in_maps = _shard_inputs(**inputs)
    if "nc" not in _CACHE:
        _CACHE["nc"] = _build_program()
    nc = _CACHE["nc"]
    trace = bool(os.environ.get("KERNEL_PROFILE")) and _install_profile_shim()
    res = run_bass_kernel_spmd(nc, in_maps, core_ids=list(range(NCORES)),
                               trace=trace)
    LAST_RUN["exec_time_ns"] = res.exec_time_ns
    LAST_RUN["mean_exec_time_ns"] = res.mean_exec_time_ns
    if res.instructions_and_trace is not None:
        LAST_RUN["trace_path"] = res.instructions_and_trace[1]
    # core c owns samples q*256 + c*32 .. +32 for q in 0..3
    out = np.empty((1, S, 1), dtype=np.float32)
    for c in range(NCORES):
        oc = np.asarray(res.results[c]["out"]).reshape(128)
        for q in range(NQ):
            out[0, q * 256 + c * 32:q * 256 + (c + 1) * 32, 0] = \
                oc[q * 32:(q + 1) * 32]
    return out


if __name__ == "__main__":
    rng = np.random.default_rng(0)
    ins = {
        "x": rng.standard_normal((T, S, G), dtype=np.float32),
        "W1": rng.standard_normal((T, G, H), dtype=np.float32) * 0.5,
        "b1": rng.standard_normal((T, G, H), dtype=np.float32) * 0.1,
        "W2": rng.standard_normal((T, G, H), dtype=np.float32) * 0.5,
        "b2": rng.standard_normal((T, G), dtype=np.float32) * 0.1,
        "Wc": rng.standard_normal((G, T), dtype=np.float32) * 0.5,
        "bc": rng.standard_normal((G,), dtype=np.float32) * 0.1,
        "CW0": rng.standard_normal((N1, G), dtype=np.float32) * 0.007,
        "Cb0": rng.standard_normal((N1,), dtype=np.float32) * 0.007,
        "CW1": rng.standard_normal((N2, N1), dtype=np.float32) * 0.02,
        "Cb1": rng.standard_normal((N2,), dtype=np.float32) * 0.02,
        "CW2": rng.standard_normal((N3, N2), dtype=np.float32) * 0.07,
        "Cb2": rng.standard_normal((N3,), dtype=np.float32) * 0.07,
        "CWf": rng.standard_normal((1, N3), dtype=np.float32) * 0.2,
        "Cbf": rng.standard_normal((1,), dtype=np.float32) * 0.2,
    }
    out = kernel(**ins)
    # numpy reference
    xx = ins["x"]
    h = np.maximum(xx[..., None] * ins["W1"][:, None] + ins["b1"][:, None], 0.0)
    y = np.einsum("tsgh,tgh->tsg", h, ins["W2"]) + ins["b2"][:, None, :]
    zz = np.einsum("tsg,gt->sg", y, ins["Wc"]) + ins["bc"]
    for Wl, bl in ((ins["CW0"], ins["Cb0"]), (ins["CW1"], ins["Cb1"]),
                   (ins["CW2"], ins["Cb2"])):
        zz = np.maximum(zz @ Wl.T + bl, 0.0)
    ref = (zz @ ins["CWf"].T + ins["Cbf"])[None]
    err = np.abs(out - ref).max() / (np.abs(ref).max() + 1e-12)
    print("self-test rel err:", err)
    print("exec_time_ns:", LAST_RUN.get("exec_time_ns"))
